# revision 1
# baseline (speedup 1.0000x reference)
"""Trainium2 Bass kernel for nn_CombinationalCircuit_31911607009919.

Computes, for a batch of B=64 candidate assignments over NV=100000 variables
and C=400000 3-SAT clauses:

    x          = sigmoid(emb_weight[input_idx])            # [B, NV]
    g          = x[:, clause_vars]                         # [B, C, 3]
    lit        = where(clause_signs > 0, g, 1 - g)
    clause_sat = 1 - prod(1 - lit, axis=-1)                # [B, C]
    out        = prod(clause_sat, axis=-1)                 # [B]

Sharding: clauses are split evenly across the 8 NeuronCores (each core keeps
all 64 batch columns).  Per core, the embedding logits for its literals are
fetched with one chunked indirect DMA gather per chunk from a variable-major
table xpm[2v + s] = (-1)^s * e_v (the literal sign is folded into the index,
using 1 - sigmoid(e) == sigmoid(-e)), then ACT computes q = sigmoid(t) =
1 - lit, DVE forms the per-clause products and a multiplicative pairwise
tree reduces them; a ones-matmul sums logs across partitions and exp yields
the core's partial product over its clause shard as [1, 64].  The partial
products are multiplied on the host (the unshard step; equivalent to the
all-reduce of log-products suggested by the sharding hint).

Padding uses a sentinel row (+/-1e30): sigmoid(-1e30) = 0 makes the padded
clause contribute sat = 1 exactly.
"""

import numpy as np

# ---------------------------------------------------------------------------
# Problem constants (hardcoded; kernel.py must be self-contained).
# ---------------------------------------------------------------------------
B = 64
NV = 100_000
C = 400_000
K = 3
NCORES = 8
P = 128

TM = 14                       # clause groups per partition per chunk
NCH = 28                      # chunks per core
# => clause slots per core = P * TM * NCH = 50176 (real: 50000)

GDT = "f32"                   # gather dtype for the logit table
BIG = 1.0e30                  # sentinel; sigmoid(-BIG) == 0 -> sat == 1

_compiled = {}                # cache so repeat calls don't recompile


def _build_bass(nch=NCH, tm=TM, repeat=1, gdt=GDT, tree=True,
                gbufs=6, qbufs=3, sbufs=4):
    import concourse.bacc as bacc
    import concourse.bass as bass
    import concourse.mybir as mybir
    import concourse.tile as tile

    f32 = mybir.dt.float32
    i32 = mybir.dt.int32
    gdtype = f32 if gdt == "f32" else mybir.dt.bfloat16
    j = K * tm                # literal columns per chunk

    nc = bacc.Bacc(
        "TRN2",
        target_bir_lowering=False,
        debug=False,
        enable_asserts=False,
        num_devices=NCORES,
    )

    # xpm[2v + s] = (-1)^s * e_v ; rows 2*NV / 2*NV+1 are +/-BIG sentinels
    xpm = nc.dram_tensor("xpm", [2 * (NV + 1), B], gdtype, kind="ExternalInput")
    idx = nc.dram_tensor("idx", [P, nch * j], i32, kind="ExternalInput")
    out = nc.dram_tensor("out", [1, B], f32, kind="ExternalOutput")

    with tile.TileContext(nc) as tc:
        with (
            tc.tile_pool(name="gath", bufs=gbufs) as gpool,
            tc.tile_pool(name="q", bufs=qbufs) as qpool,
            tc.tile_pool(name="sat", bufs=sbufs) as spool,
            tc.tile_pool(name="const", bufs=1) as cpool,
            tc.tile_pool(name="ps", bufs=1, space="PSUM") as pspool,
        ):
            # all chunk indices resident: ix_all[p, ch*j + jj]
            ix_all = cpool.tile([P, nch * j], i32, tag="ixall")
            nc.sync.dma_start(out=ix_all[:], in_=idx[:])
            ones = cpool.tile([P, 1], f32)
            nc.vector.memset(ones[:], 1.0)

            for _rep in range(repeat):
              acc = cpool.tile([P, B], f32, tag="acc")
              nc.vector.memset(acc[:], 1.0 if tree else 0.0)
              for ch in range(nch):
                # gather: G[p, jj*B + b] = xpm[ix[p, jj], b].
                # The HW indirect-DGE consumes ONE offset per output
                # partition-row (desc length = innermost dest extent), so
                # gather 128 rows per call: offsets [P, 1] -> dest [P, B].
                G = gpool.tile([P, j * B], gdtype)
                for t in range(j):
                    nc.gpsimd.indirect_dma_start(
                        out=G[:, t * B:(t + 1) * B],
                        out_offset=None,
                        in_=xpm[:],
                        in_offset=bass.IndirectOffsetOnAxis(
                            ap=ix_all[:, ch * j + t:ch * j + t + 1], axis=0
                        ),
                    )

                # q = sigmoid(t) = 1 - lit   (upcast to f32 if gathering bf16)
                Q = G if gdt == "f32" else qpool.tile([P, j * B], f32)
                nc.scalar.activation(
                    Q[:], G[:], mybir.ActivationFunctionType.Sigmoid
                )

                # per-clause product u = q0*q1*q2, then sat = 1 - u
                Qk = Q.rearrange("p (m k b) -> p m k b", k=K, b=B)
                Ssat = spool.tile([P, tm * B], f32)
                S3 = Ssat.rearrange("p (m b) -> p m b", b=B)
                nc.vector.tensor_tensor(
                    out=S3,
                    in0=Qk[:, :, 0, :],
                    in1=Qk[:, :, 1, :],
                    op=mybir.AluOpType.mult,
                )
                nc.vector.tensor_tensor(
                    out=S3, in0=S3, in1=Qk[:, :, 2, :], op=mybir.AluOpType.mult
                )
                nc.vector.tensor_scalar(
                    out=Ssat[:],
                    in0=Ssat[:],
                    scalar1=-1.0,
                    scalar2=1.0,
                    op0=mybir.AluOpType.mult,
                    op1=mybir.AluOpType.add,
                )

                if tree:
                    # multiplicative pairwise tree over the tm clause groups
                    # (contiguous DVE ops), folded into the running product
                    n = tm
                    while n > 1:
                        if n % 2 == 1:
                            nc.vector.tensor_tensor(
                                out=Ssat[:, :B],
                                in0=Ssat[:, :B],
                                in1=Ssat[:, (n - 1) * B:n * B],
                                op=mybir.AluOpType.mult,
                            )
                            n -= 1
                            continue
                        h = n // 2
                        nc.vector.tensor_tensor(
                            out=Ssat[:, :h * B],
                            in0=Ssat[:, :h * B],
                            in1=Ssat[:, h * B:n * B],
                            op=mybir.AluOpType.mult,
                        )
                        n = h
                    nc.vector.tensor_tensor(
                        out=acc[:], in0=acc[:], in1=Ssat[:, :B],
                        op=mybir.AluOpType.mult,
                    )
                else:
                    # ln(clause_sat), then strided sum over clause groups
                    nc.scalar.activation(
                        Ssat[:], Ssat[:], mybir.ActivationFunctionType.Ln
                    )
                    R = spool.tile([P, B], f32, tag="red")
                    nc.vector.tensor_reduce(
                        out=R[:],
                        in_=Ssat.rearrange("p (m b) -> p b m", b=B),
                        op=mybir.AluOpType.add,
                        axis=mybir.AxisListType.X,
                    )
                    nc.vector.tensor_tensor(
                        out=acc[:], in0=acc[:], in1=R[:],
                        op=mybir.AluOpType.add,
                    )

            # log of per-partition partials, summed across partitions with a
            # ones-matmul, then exp -> partial product over the clause shard.
            if tree:
                nc.scalar.activation(
                    acc[:], acc[:], mybir.ActivationFunctionType.Ln
                )
            psum = pspool.tile([1, B], f32)
            nc.tensor.matmul(psum[:], lhsT=ones[:], rhs=acc[:], start=True, stop=True)
            res = cpool.tile([1, B], f32, tag="res")
            nc.scalar.activation(
                res[:], psum[:], mybir.ActivationFunctionType.Exp
            )
            nc.sync.dma_start(out=out[:], in_=res[:])

    nc.compile()
    return nc


def _get_compiled(nch=NCH, tm=TM, repeat=1, gdt=GDT, tree=True):
    key = (nch, tm, repeat, gdt, tree)
    if key not in _compiled:
        _compiled[key] = _build_bass(nch, tm, repeat, gdt, tree)
    return _compiled[key]


def _np_gdt(gdt):
    if gdt == "f32":
        return np.float32
    import ml_dtypes
    return ml_dtypes.bfloat16


def _make_table(input_idx, emb_weight, gdt=GDT):
    """xpm[2v + s] = (-1)^s * e_v  as [2*(NV+1), B]; rows 2NV, 2NV+1 are the
    +/-BIG padding sentinels."""
    input_idx = np.asarray(input_idx)
    emb_weight = np.asarray(emb_weight, dtype=np.float32)
    xrows = emb_weight[input_idx.astype(np.int64)]          # [B, NV]
    xpm = np.empty((NV + 1, 2, B), dtype=_np_gdt(gdt))
    xpm[:NV, 0, :] = xrows.T
    xpm[NV, 0, :] = BIG
    xpm[:, 1, :] = -xpm[:, 0, :]
    return np.ascontiguousarray(xpm.reshape(2 * (NV + 1), B))


def _shard_clauses(clause_vars, clause_signs, nch=NCH, tm=TM, s_core=None):
    """Split clauses into NCORES shards, pad to P*tm*nch slots, and build the
    per-core sign-folded index arrays: idx = 2*v + (sign > 0)."""
    clause_vars = np.asarray(clause_vars)
    clause_signs = np.asarray(clause_signs)
    sp = P * tm * nch
    if s_core is None:
        s_core = sp if len(clause_vars) >= sp * NCORES else len(clause_vars) // NCORES
    j = K * tm

    shards = []
    for core in range(NCORES):
        lo = core * s_core
        v = clause_vars[lo:lo + s_core]
        s = clause_signs[lo:lo + s_core]

        # pad -> sentinel row 2*NV+1 (= -BIG -> q = 0 -> sat = 1)
        ip = np.full((sp, K), 2 * NV + 1, dtype=np.int32)
        ip[:len(v)] = 2 * v.astype(np.int64) + (s > 0)

        # clause slot (ch, p, m): c_local = ch*(P*tm) + p*tm + m
        # stored partition-major: idx[p, ch*j + jj]
        idx_host = np.ascontiguousarray(
            ip.reshape(nch, P, j).transpose(1, 0, 2).reshape(P, nch * j)
        )                                                   # [P, nch*j]
        shards.append(idx_host)
    return shards


def _prepare_inputs(input_idx, emb_weight, clause_vars, clause_signs,
                    nch=NCH, tm=TM, s_core=None, gdt=GDT):
    xpm = _make_table(input_idx, emb_weight, gdt)
    shards = _shard_clauses(clause_vars, clause_signs, nch, tm, s_core)
    return [{"xpm": xpm, "idx": ih} for ih in shards]


def _run(in_maps, nch=NCH, tm=TM, gdt=GDT, tree=True, trace=False):
    from concourse.bass_utils import run_bass_kernel_spmd

    nc = _get_compiled(nch, tm, 1, gdt, tree)
    return run_bass_kernel_spmd(
        nc, in_maps, core_ids=list(range(NCORES)), trace=trace
    )


def kernel(input_idx, emb_weight, clause_vars, clause_signs):
    in_maps = _prepare_inputs(input_idx, emb_weight, clause_vars, clause_signs)
    results = _run(in_maps)
    partials = np.stack(
        [np.asarray(r["out"]).reshape(B) for r in results.results]
    )                                                       # [NCORES, B]
    # combine the per-shard partial products (all-reduce of log-products)
    return np.prod(partials, axis=0).astype(np.float32)

